# revision 16
# baseline (speedup 1.0000x reference)
"""GraphSAGE edge layer (sigmoid-gated message passing + segment-max) on 8 Trainium2
NeuronCores via Bass/Tile.

Strategy (graph/data parallel):
  - Nodes are sharded across 8 cores (6250 each); edges partitioned by destination
    node so the segment-max reduce is core-local.
  - Per core, a replicated G table [Ah | Bh] (bf16) is built on-device in a
    partition-major layout: node n lives at row (n%128)*NBLK1 + n//128, so each
    partition's G rows are contiguous and phase-1 writes are large descriptors.
  - Edges are packed host-side into a degree-bucketed (node x slot) grid: 49 blocks
    of 128 nodes, block b has K_b slots/node (max in-degree in block, shared across
    cores so the SPMD program has one shape schedule). Pad slots point at a zero row
    of G, whose Ah half is 0 => gated contribution is exactly 0, which is absorbed
    by the final clamp-to-0 (relu of the segment max).
  - Per block: ONE indirect DMA gathers all slot rows (+ the block's own rows for
    Bh[dst]); VectorE/ScalarE compute sigmoid(BhS + BhD) * AhS; the segment max is
    a contiguous in-place tree of tensor_tensor(max) ops (2x bf16 DVE mode); the
    update MLP runs fused per block in f32; sqrt/normalize/residual for all blocks
    run in one deferred tail pass (single activation-table load, batched stores).
"""

import numpy as np
import ml_dtypes

from concourse import bass, bacc, mybir
from concourse.tile import TileContext
from concourse.bass_utils import run_bass_kernel_spmd
from concourse.masks import make_identity

BF16 = ml_dtypes.bfloat16

N = 50000
E = 800000
D = 128
NCORES = 8
NLOC = N // NCORES          # 6250 nodes per core
P = 128
NBLK1 = (N + P - 1) // P    # 391 G-build blocks
NPADG = NBLK1 * P           # 50048 (pad/zero row of G lives at this index)
NBLK = (NLOC + P - 1) // P  # 49 local node blocks
NLOCP = NBLK * P            # 6272
EPS = 1e-12
CH1 = 18                    # G-build hT chunk (blocks per DMA)
PC1 = 6                     # G-build PSUM chunk (blocks per PSUM tile)
OW = 8                      # output write chunk (blocks per DMA)

_prog_cache = {}


# --------------------------------------------------------------------------- host


def _rowmap(n):
    """Node id -> partition-major G row."""
    return (n % P) * NBLK1 + n // P


def _preprocess(h, src, dst):
    """Shard edges by destination, build per-core degree-sorted slot grids.

    Returns (K, offs, total_cols, per_core) where per_core[c] =
    (gidx [128, total_cols] int32, perm [NLOC] int64). gidx entries are
    partition-major G row indices (or NPADG for pad slots).
    """
    src = np.asarray(src).astype(np.int64)
    dst = np.asarray(dst).astype(np.int64)

    order = np.argsort(dst, kind="stable")
    dst_s = dst[order]
    src_s = src[order]
    bounds = np.searchsorted(dst_s, np.arange(NCORES + 1) * NLOC)

    cores = []
    blkmax = np.zeros((NCORES, NBLK), np.int64)
    for c in range(NCORES):
        a, b = bounds[c], bounds[c + 1]
        ldst = dst_s[a:b] - c * NLOC
        lsrc = src_s[a:b]
        deg = np.bincount(ldst, minlength=NLOC)
        perm = np.argsort(-deg, kind="stable")
        pdeg = np.zeros(NLOCP, np.int64)
        pdeg[:NLOC] = deg[perm]
        blkmax[c] = pdeg.reshape(NBLK, P).max(axis=1)
        cores.append((lsrc, deg, perm))

    K = np.maximum(blkmax.max(axis=0), 1).astype(np.int64)
    offs = np.concatenate([[0], np.cumsum(K + 1)]).astype(np.int64)
    total_cols = int(offs[-1])

    per_core = []
    for c in range(NCORES):
        lsrc, deg, perm = cores[c]
        start = np.concatenate([[0], np.cumsum(deg)])
        gidx = np.full((P, total_cols), NPADG, np.int32)
        for blk in range(NBLK):
            o = int(offs[blk])
            kb = int(K[blk])
            for p in range(P):
                i = blk * P + p
                if i >= NLOC:
                    continue
                node = int(perm[i])
                d = int(deg[node])
                if d:
                    e = lsrc[start[node] : start[node] + d]
                    gidx[p, o : o + d] = _rowmap(e)
                gidx[p, o + kb] = _rowmap(c * NLOC + node)
        per_core.append((gidx, perm))
    return K, offs, total_cols, per_core


# --------------------------------------------------------------------------- bass


def _build(K, offs, total_cols):
    f32 = mybir.dt.float32
    bf16 = mybir.dt.bfloat16
    i32 = mybir.dt.int32

    nc = bacc.Bacc(
        "TRN2", target_bir_lowering=False, debug=False, num_devices=NCORES
    )
    hT = nc.declare_dram_parameter("hT", [D, NPADG], bf16, isOutput=False)
    # hloc is partition-major: [p, blk, x]
    hloc = nc.declare_dram_parameter("hloc", [P, NBLK * D], f32, isOutput=False)
    hTloc = nc.declare_dram_parameter("hTloc", [D, NLOCP], f32, isOutput=False)
    gidx = nc.declare_dram_parameter("gidx", [P, total_cols], i32, isOutput=False)
    wcat = nc.declare_dram_parameter("wcat", [D, 2 * D], bf16, isOutput=False)
    brhs = nc.declare_dram_parameter("brhs", [1, PC1 * 2 * D], f32, isOutput=False)
    u1 = nc.declare_dram_parameter("u1", [D, D], f32, isOutput=False)
    u2 = nc.declare_dram_parameter("u2", [D, D], f32, isOutput=False)
    ubr = nc.declare_dram_parameter("ubr", [1, D], f32, isOutput=False)
    # out is partition-major: [p, blk, x]
    outp = nc.declare_dram_parameter("out", [P, NBLK * D], f32, isOutput=True)

    G = nc.dram_tensor("G", [NPADG + 1, 2 * D], bf16)
    # partition-major view of the first NPADG rows: row (p*NBLK1 + j)
    Gp = G[0:NPADG, :].rearrange("(p j) x -> p j x", p=P)

    with TileContext(nc) as tc:
        with (
            tc.tile_pool(name="const", bufs=1) as cpool,
            tc.tile_pool(name="ht1", bufs=2) as htpool,
            tc.tile_pool(name="gs1", bufs=2) as gspool,
            tc.tile_pool(name="gat", bufs=2) as gatpool,
            tc.tile_pool(name="sg", bufs=2) as sgpool,
            tc.tile_pool(name="p3", bufs=3) as p3pool,
            tc.tile_pool(name="ow", bufs=2) as owpool,
            tc.tile_pool(name="ps1", bufs=2, space="PSUM") as ps1pool,
            tc.tile_pool(name="ps3", bufs=2, space="PSUM") as ps3pool,
        ):
            # ---- constants
            wcat_t = cpool.tile([D, 2 * D], bf16)
            nc.sync.dma_start(out=wcat_t[:], in_=wcat[:, :])
            brhs_t = cpool.tile([1, PC1 * 2 * D], f32)
            nc.sync.dma_start(out=brhs_t[:], in_=brhs[:, :])
            u1_t = cpool.tile([D, D], f32)
            nc.sync.dma_start(out=u1_t[:], in_=u1[:, :])
            u2_t = cpool.tile([D, D], f32)
            nc.sync.dma_start(out=u2_t[:], in_=u2[:, :])
            ubr_t = cpool.tile([1, D], f32)
            nc.sync.dma_start(out=ubr_t[:], in_=ubr[:, :])
            ident = cpool.tile([P, P], f32)
            make_identity(nc, ident[:])

            # bias rows replicated across partitions (matmul ones-trick)
            ones_f32 = cpool.tile([1, P], f32)
            nc.vector.memset(ones_f32[:], 1.0)
            bias_ps = ps1pool.tile([P, PC1 * 2 * D], f32, tag="gps")
            for q in range(0, PC1 * 2 * D, 512):
                nc.tensor.matmul(
                    out=bias_ps[:, q : q + 512],
                    lhsT=ones_f32[:],
                    rhs=brhs_t[:, q : q + 512],
                    start=True,
                    stop=True,
                )
            bias6_sb = cpool.tile([P, PC1 * 2 * D], f32)
            nc.scalar.copy(out=bias6_sb[:], in_=bias_ps[:])
            ub_ps = ps1pool.tile([P, PC1 * 2 * D], f32, tag="gps")
            nc.tensor.matmul(
                out=ub_ps[:, 0:D], lhsT=ones_f32[:], rhs=ubr_t[:], start=True, stop=True
            )
            ub_sb = cpool.tile([P, D], f32)
            nc.scalar.copy(out=ub_sb[:], in_=ub_ps[:, 0:D])

            zrow = cpool.tile([1, 2 * D], bf16)
            nc.vector.memset(zrow[:], 0.0)
            nc.sync.dma_start(out=G[NPADG : NPADG + 1, :], in_=zrow[:])

            # ---- phase 1: G = [h @ A_w + A_b | h @ B_w + B_b]  (bf16)
            for c0 in range(0, NBLK1, CH1):
                nb = min(CH1, NBLK1 - c0)
                ht_t = htpool.tile([D, CH1 * P], bf16, tag="ht")
                nc.sync.dma_start(
                    out=ht_t[:, : nb * P],
                    in_=hT[:, c0 * P : (c0 + nb) * P],
                )
                gs = gspool.tile([P, CH1 * 2 * D], bf16, tag="gs")
                for g0 in range(0, nb, PC1):
                    gn = min(PC1, nb - g0)
                    ps = ps1pool.tile([P, PC1 * 2 * D], f32, tag="gps")
                    for j in range(g0, g0 + gn):
                        nc.tensor.matmul(
                            out=ps[:, (j - g0) * 2 * D : (j - g0 + 1) * 2 * D],
                            lhsT=ht_t[:, j * P : (j + 1) * P],
                            rhs=wcat_t[:],
                            start=True,
                            stop=True,
                        )
                    # chunked bias add fused with PSUM -> SBUF move (VectorE)
                    nc.vector.tensor_tensor(
                        out=gs[:, g0 * 2 * D : (g0 + gn) * 2 * D],
                        in0=ps[:, : gn * 2 * D],
                        in1=bias6_sb[:, : gn * 2 * D],
                        op=mybir.AluOpType.add,
                    )
                nc.sync.dma_start(
                    out=Gp[:, c0 : c0 + nb, :],
                    in_=gs[:, : nb * 2 * D].rearrange("p (j x) -> p j x", j=nb),
                )

            # phase-2/3 preloads (issued late so phase-1 hT DMAs go first)
            idx_t = cpool.tile([P, total_cols], i32)
            nc.sync.dma_start(out=idx_t[:], in_=gidx[:, :])
            hloc_sb = cpool.tile([P, NBLK * D], f32)
            nc.sync.dma_start(out=hloc_sb[:], in_=hloc[:, :])
            hTloc_sb = cpool.tile([D, NLOCP], f32)
            nc.sync.dma_start(out=hTloc_sb[:], in_=hTloc[:, :])

            # collapse phase-1 -> phase-2 deps into one barrier (sync-wait
            # slots on a single instruction are limited)
            tc.strict_bb_all_engine_barrier()

            # persistent phase-3 state (finished in overlapping sub-tails)
            bun_all = cpool.tile([P, NBLK * D], f32)
            ssq_all = cpool.tile([P, NBLK], f32)
            nrm_all = cpool.tile([P, NBLK], f32)
            rn_all = cpool.tile([P, NBLK], f32)

            _tail_done = [0]

            def _subtail(upto):
                t0 = _tail_done[0]
                if upto <= t0:
                    return
                nc.scalar.activation(
                    out=nrm_all[:, t0:upto],
                    in_=ssq_all[:, t0:upto],
                    func=mybir.ActivationFunctionType.Sqrt,
                )
                nc.vector.tensor_scalar_max(
                    nrm_all[:, t0:upto], nrm_all[:, t0:upto], EPS
                )
                nc.vector.reciprocal(rn_all[:, t0:upto], nrm_all[:, t0:upto])
                for b0 in range(t0, upto, OW):
                    nb = min(OW, upto - b0)
                    ow = owpool.tile([P, OW * D], f32, tag="ow")
                    for i in range(nb):
                        blk = b0 + i
                        ob = ow[:, i * D : (i + 1) * D]
                        nc.vector.tensor_scalar(
                            out=ob,
                            in0=bun_all[:, blk * D : (blk + 1) * D],
                            scalar1=rn_all[:, blk : blk + 1],
                            scalar2=0.0,
                            op0=mybir.AluOpType.mult,
                            op1=mybir.AluOpType.max,
                        )
                        nc.vector.tensor_tensor(
                            out=ob,
                            in0=ob,
                            in1=hloc_sb[:, blk * D : (blk + 1) * D],
                            op=mybir.AluOpType.add,
                        )
                    nc.sync.dma_start(
                        out=outp[:, b0 * D : (b0 + nb) * D],
                        in_=ow[:, : nb * D],
                    )
                _tail_done[0] = upto

            # ---- phase 2+3 per local node block
            for blk in range(NBLK):
                kb = int(K[blk])
                o = int(offs[blk])

                gt = gatpool.tile([P, (kb + 1) * 2 * D], bf16, tag="gt")
                nc.gpsimd.indirect_dma_start(
                    out=gt[:],
                    out_offset=None,
                    in_=G[:, :],
                    in_offset=bass.IndirectOffsetOnAxis(
                        ap=idx_t[:, o : o + kb + 1], axis=0
                    ),
                )

                gv = gt[:, : kb * 2 * D].rearrange("p (k x) -> p k x", k=kb)
                ahs = gv[:, :, 0:D]
                bhs = gv[:, :, D : 2 * D]
                bhd = (
                    gt[:, kb * 2 * D + D : (kb + 1) * 2 * D]
                    .rearrange("p (o x) -> p o x", o=1)
                    .to_broadcast([P, kb, D])
                )

                s = sgpool.tile([P, kb * D], bf16, tag="s")
                s3 = s[:].rearrange("p (k x) -> p k x", k=kb)
                nc.vector.tensor_tensor(
                    out=s3, in0=bhs, in1=bhd, op=mybir.AluOpType.add
                )
                nc.scalar.activation(
                    out=s[:], in_=s[:], func=mybir.ActivationFunctionType.Sigmoid
                )
                nc.vector.tensor_tensor(
                    out=s3, in0=s3, in1=ahs, op=mybir.AluOpType.mult
                )

                # segment max: contiguous in-place tree of tensor_tensor(max)
                cb = p3pool.tile([P, D], f32, tag="cb")
                n = kb
                while n > 1:
                    if n & 1:
                        # fold the tail column into column 0, making n even
                        nc.vector.tensor_tensor(
                            out=s[:, 0:D],
                            in0=s[:, 0:D],
                            in1=s[:, (n - 1) * D : n * D],
                            op=mybir.AluOpType.max,
                        )
                        n -= 1
                        if n == 1:
                            break
                    h = n // 2
                    if h == 1:
                        break
                    nc.vector.tensor_tensor(
                        out=s[:, : h * D],
                        in0=s[:, : h * D],
                        in1=s[:, h * D : n * D],
                        op=mybir.AluOpType.max,
                    )
                    n = h
                if n == 1:
                    nc.vector.tensor_scalar_max(cb[:], s[:, 0:D], 0.0)
                else:
                    # cb = max(s0, s1, 0): relu folded into the last tree op
                    nc.vector.scalar_tensor_tensor(
                        out=cb[:],
                        in0=s[:, 0:D],
                        scalar=0.0,
                        in1=s[:, D : 2 * D],
                        op0=mybir.AluOpType.max,
                        op1=mybir.AluOpType.max,
                    )

                # ---- phase 3 (fused): bundle = h @ U1 + c @ U2 + U_b
                p3ps = ps3pool.tile([P, 2 * D], f32, tag="p3ps")
                ct_ps = p3ps[:, 0:D]
                bp = p3ps[:, D : 2 * D]
                nc.tensor.transpose(out=ct_ps, in_=cb[:], identity=ident[:])
                ct = p3pool.tile([P, D], f32, tag="ct")
                nc.scalar.copy(out=ct[:], in_=ct_ps)

                nc.tensor.matmul(
                    out=bp,
                    lhsT=hTloc_sb[:, blk * P : (blk + 1) * P],
                    rhs=u1_t[:],
                    start=True,
                    stop=False,
                )
                nc.tensor.matmul(
                    out=bp, lhsT=ct[:], rhs=u2_t[:], start=False, stop=True
                )

                # bundle = bp + U_b, moved to SBUF (VectorE)
                bun = bun_all[:, blk * D : (blk + 1) * D]
                nc.vector.tensor_tensor(
                    out=bun, in0=bp, in1=ub_sb[:], op=mybir.AluOpType.add
                )
                # sum of squares (ScalarE Square is in the sigmoid table set)
                sq = p3pool.tile([P, D], f32, tag="sq")
                nc.scalar.activation(
                    out=sq[:],
                    in_=bun,
                    func=mybir.ActivationFunctionType.Square,
                    accum_out=ssq_all[:, blk : blk + 1],
                )
                if blk in (15, 31):
                    _subtail(blk + 1)

            # ---- final sub-tail flush
            _subtail(NBLK)

    nc.compile()
    return nc


# --------------------------------------------------------------------------- run


def _run(inputs, trace=False):
    h = np.asarray(inputs["h"], np.float32)
    A_w = np.asarray(inputs["A_w"], np.float32)
    A_b = np.asarray(inputs["A_b"], np.float32)
    B_w = np.asarray(inputs["B_w"], np.float32)
    B_b = np.asarray(inputs["B_b"], np.float32)
    U_w = np.asarray(inputs["U_w"], np.float32)
    U_b = np.asarray(inputs["U_b"], np.float32)

    K, offs, total_cols, per_core = _preprocess(h, inputs["src"], inputs["dst"])

    key = (tuple(int(k) for k in K), total_cols)
    if key not in _prog_cache:
        _prog_cache.clear()
        _prog_cache[key] = _build(K, offs, total_cols)
    nc = _prog_cache[key]

    hT_bf = np.zeros((D, NPADG), BF16)
    hT_bf[:, :N] = h.T.astype(BF16)
    wcat = np.concatenate([A_w, B_w], axis=1).astype(BF16)
    brhs = np.tile(np.concatenate([A_b, B_b]), 6)[None, :].astype(np.float32)
    u1 = np.ascontiguousarray(U_w[:D]).astype(np.float32)
    u2 = np.ascontiguousarray(U_w[D:]).astype(np.float32)
    ubr = U_b[None, :].astype(np.float32)

    in_maps = []
    for c in range(NCORES):
        gidx_c, perm = per_core[c]
        hl = np.zeros((NLOCP, D), np.float32)
        hl[:NLOC] = h[c * NLOC + perm]
        # partition-major [p, blk, x] layout for contiguous per-partition DMA
        hl_p = np.ascontiguousarray(
            hl.reshape(NBLK, P, D).transpose(1, 0, 2).reshape(P, NBLK * D)
        )
        in_maps.append(
            {
                "hT": hT_bf,
                "hloc": hl_p,
                "hTloc": np.ascontiguousarray(hl.T),
                "gidx": gidx_c,
                "wcat": wcat,
                "brhs": brhs,
                "u1": u1,
                "u2": u2,
                "ubr": ubr,
            }
        )

    res = run_bass_kernel_spmd(nc, in_maps, list(range(NCORES)), trace=trace)

    out = np.empty((N, D), np.float32)
    for c in range(NCORES):
        _, perm = per_core[c]
        # undo partition-major layout: res [p, blk, x] -> [blk*P + p, x]
        o = (
            res.results[c]["out"]
            .reshape(P, NBLK, D)
            .transpose(1, 0, 2)
            .reshape(NLOCP, D)
        )
        out[c * NLOC + perm] = o[:NLOC]
    return out, res


def kernel(**inputs) -> np.ndarray:
    out, _ = _run(inputs, trace=False)
    return out


# revision 17
# speedup vs baseline: 1.1104x; 1.1104x over previous
"""GraphSAGE edge layer (sigmoid-gated message passing + segment-max) on 8 Trainium2
NeuronCores via Bass/Tile.

Strategy (graph/data parallel):
  - Nodes are sharded across 8 cores (6250 each); edges partitioned by destination
    node so the segment-max reduce is core-local.
  - Per core, a replicated G table [Ah | Bh] (bf16) is built on-device in a
    partition-major layout: node n lives at row (n%128)*NBLK1 + n//128, so each
    partition's G rows are contiguous and phase-1 writes are large descriptors.
  - Edges are packed host-side into a degree-bucketed (node x slot) grid: 49 blocks
    of 128 nodes, block b has K_b slots/node (max in-degree in block, shared across
    cores so the SPMD program has one shape schedule). Pad slots point at a zero row
    of G, whose Ah half is 0 => gated contribution is exactly 0, which is absorbed
    by the final clamp-to-0 (relu of the segment max).
  - Per block: ONE indirect DMA gathers all slot rows (+ the block's own rows for
    Bh[dst]); VectorE/ScalarE compute sigmoid(BhS + BhD) * AhS; the segment max is
    a contiguous in-place tree of tensor_tensor(max) ops (2x bf16 DVE mode); the
    update MLP runs fused per block in f32; sqrt/normalize/residual for all blocks
    run in one deferred tail pass (single activation-table load, batched stores).
"""

import numpy as np
import ml_dtypes

from concourse import bass, bacc, mybir
from concourse.tile import TileContext
from concourse.bass_utils import run_bass_kernel_spmd
from concourse.masks import make_identity

BF16 = ml_dtypes.bfloat16

N = 50000
E = 800000
D = 128
NCORES = 8
NLOC = N // NCORES          # 6250 nodes per core
P = 128
NBLK1 = (N + P - 1) // P    # 391 G-build blocks
NPADG = NBLK1 * P           # 50048 (pad/zero row of G lives at this index)
NBLK = (NLOC + P - 1) // P  # 49 local node blocks
NLOCP = NBLK * P            # 6272
EPS = 1e-12
CH1 = 16                    # G-build hT chunk (blocks per DMA)
PC1 = 4                     # G-build PSUM chunk (blocks per PSUM tile)
OW = 8                      # output write chunk (blocks per DMA)

_prog_cache = {}


# --------------------------------------------------------------------------- host


def _rowmap(n):
    """Node id -> partition-major G row."""
    return (n % P) * NBLK1 + n // P


def _preprocess(h, src, dst):
    """Shard edges by destination, build per-core degree-sorted slot grids.

    Returns (K, offs, total_cols, per_core) where per_core[c] =
    (gidx [128, total_cols] int32, perm [NLOC] int64). gidx entries are
    partition-major G row indices (or NPADG for pad slots).
    """
    src = np.asarray(src).astype(np.int64)
    dst = np.asarray(dst).astype(np.int64)

    order = np.argsort(dst, kind="stable")
    dst_s = dst[order]
    src_s = src[order]
    bounds = np.searchsorted(dst_s, np.arange(NCORES + 1) * NLOC)

    cores = []
    blkmax = np.zeros((NCORES, NBLK), np.int64)
    for c in range(NCORES):
        a, b = bounds[c], bounds[c + 1]
        ldst = dst_s[a:b] - c * NLOC
        lsrc = src_s[a:b]
        deg = np.bincount(ldst, minlength=NLOC)
        perm = np.argsort(-deg, kind="stable")
        pdeg = np.zeros(NLOCP, np.int64)
        pdeg[:NLOC] = deg[perm]
        blkmax[c] = pdeg.reshape(NBLK, P).max(axis=1)
        cores.append((lsrc, deg, perm))

    K = np.maximum(blkmax.max(axis=0), 1).astype(np.int64)
    offs = np.concatenate([[0], np.cumsum(K + 1)]).astype(np.int64)
    total_cols = int(offs[-1])

    per_core = []
    for c in range(NCORES):
        lsrc, deg, perm = cores[c]
        start = np.concatenate([[0], np.cumsum(deg)])
        gidx = np.full((P, total_cols), NPADG, np.int32)
        for blk in range(NBLK):
            o = int(offs[blk])
            kb = int(K[blk])
            for p in range(P):
                i = blk * P + p
                if i >= NLOC:
                    continue
                node = int(perm[i])
                d = int(deg[node])
                if d:
                    e = lsrc[start[node] : start[node] + d]
                    gidx[p, o : o + d] = _rowmap(e)
                gidx[p, o + kb] = _rowmap(c * NLOC + node)
        per_core.append((gidx, perm))
    return K, offs, total_cols, per_core


# --------------------------------------------------------------------------- bass


def _build(K, offs, total_cols):
    f32 = mybir.dt.float32
    bf16 = mybir.dt.bfloat16
    i32 = mybir.dt.int32

    nc = bacc.Bacc(
        "TRN2", target_bir_lowering=False, debug=False, num_devices=NCORES
    )
    hT = nc.declare_dram_parameter("hT", [D, NPADG], bf16, isOutput=False)
    # hloc is partition-major: [p, blk, x]
    hloc = nc.declare_dram_parameter("hloc", [P, NBLK * D], f32, isOutput=False)
    hTloc = nc.declare_dram_parameter("hTloc", [D, NLOCP], f32, isOutput=False)
    gidx = nc.declare_dram_parameter("gidx", [P, total_cols], i32, isOutput=False)
    wcat = nc.declare_dram_parameter("wcat", [D, 2 * D], bf16, isOutput=False)
    brhs = nc.declare_dram_parameter("brhs", [1, PC1 * 2 * D], f32, isOutput=False)
    u1 = nc.declare_dram_parameter("u1", [D, D], f32, isOutput=False)
    u2 = nc.declare_dram_parameter("u2", [D, D], f32, isOutput=False)
    ubr = nc.declare_dram_parameter("ubr", [1, D], f32, isOutput=False)
    # out is partition-major: [p, blk, x]
    outp = nc.declare_dram_parameter("out", [P, NBLK * D], f32, isOutput=True)

    G = nc.dram_tensor("G", [NPADG + 1, 2 * D], bf16)
    # partition-major view of the first NPADG rows: row (p*NBLK1 + j)
    Gp = G[0:NPADG, :].rearrange("(p j) x -> p j x", p=P)

    with TileContext(nc) as tc:
        with (
            tc.tile_pool(name="const", bufs=1) as cpool,
            tc.tile_pool(name="ht1", bufs=2) as htpool,
            tc.tile_pool(name="gs1", bufs=2) as gspool,
            tc.tile_pool(name="gat", bufs=2) as gatpool,
            tc.tile_pool(name="sg", bufs=2) as sgpool,
            tc.tile_pool(name="p3", bufs=3) as p3pool,
            tc.tile_pool(name="ow", bufs=2) as owpool,
            tc.tile_pool(name="ps1", bufs=2, space="PSUM") as ps1pool,
            tc.tile_pool(name="ps3", bufs=2, space="PSUM") as ps3pool,
        ):
            # ---- constants
            wcat_t = cpool.tile([D, 2 * D], bf16)
            nc.sync.dma_start(out=wcat_t[:], in_=wcat[:, :])
            brhs_t = cpool.tile([1, PC1 * 2 * D], f32)
            nc.sync.dma_start(out=brhs_t[:], in_=brhs[:, :])
            u1_t = cpool.tile([D, D], f32)
            nc.sync.dma_start(out=u1_t[:], in_=u1[:, :])
            u2_t = cpool.tile([D, D], f32)
            nc.sync.dma_start(out=u2_t[:], in_=u2[:, :])
            ubr_t = cpool.tile([1, D], f32)
            nc.sync.dma_start(out=ubr_t[:], in_=ubr[:, :])
            ident = cpool.tile([P, P], f32)
            make_identity(nc, ident[:])

            # bias rows replicated across partitions (matmul ones-trick)
            ones_f32 = cpool.tile([1, P], f32)
            nc.vector.memset(ones_f32[:], 1.0)
            bias_ps = ps1pool.tile([P, PC1 * 2 * D], f32, tag="gps")
            for q in range(0, PC1 * 2 * D, 512):
                nc.tensor.matmul(
                    out=bias_ps[:, q : q + 512],
                    lhsT=ones_f32[:],
                    rhs=brhs_t[:, q : q + 512],
                    start=True,
                    stop=True,
                )
            bias6_sb = cpool.tile([P, PC1 * 2 * D], f32)
            nc.scalar.copy(out=bias6_sb[:], in_=bias_ps[:])
            ub_ps = ps1pool.tile([P, PC1 * 2 * D], f32, tag="gps")
            nc.tensor.matmul(
                out=ub_ps[:, 0:D], lhsT=ones_f32[:], rhs=ubr_t[:], start=True, stop=True
            )
            ub_sb = cpool.tile([P, D], f32)
            nc.scalar.copy(out=ub_sb[:], in_=ub_ps[:, 0:D])

            zrow = cpool.tile([1, 2 * D], bf16)
            nc.vector.memset(zrow[:], 0.0)
            nc.sync.dma_start(out=G[NPADG : NPADG + 1, :], in_=zrow[:])

            # ---- phase 1: G = [h @ A_w + A_b | h @ B_w + B_b]  (bf16)
            for c0 in range(0, NBLK1, CH1):
                nb = min(CH1, NBLK1 - c0)
                ht_t = htpool.tile([D, CH1 * P], bf16, tag="ht")
                nc.sync.dma_start(
                    out=ht_t[:, : nb * P],
                    in_=hT[:, c0 * P : (c0 + nb) * P],
                )
                gs = gspool.tile([P, CH1 * 2 * D], bf16, tag="gs")
                for g0 in range(0, nb, PC1):
                    gn = min(PC1, nb - g0)
                    ps = ps1pool.tile([P, PC1 * 2 * D], f32, tag="gps")
                    for j in range(g0, g0 + gn):
                        nc.tensor.matmul(
                            out=ps[:, (j - g0) * 2 * D : (j - g0 + 1) * 2 * D],
                            lhsT=ht_t[:, j * P : (j + 1) * P],
                            rhs=wcat_t[:],
                            start=True,
                            stop=True,
                        )
                    # chunked bias add fused with PSUM -> SBUF move (VectorE)
                    nc.vector.tensor_tensor(
                        out=gs[:, g0 * 2 * D : (g0 + gn) * 2 * D],
                        in0=ps[:, : gn * 2 * D],
                        in1=bias6_sb[:, : gn * 2 * D],
                        op=mybir.AluOpType.add,
                    )
                nc.sync.dma_start(
                    out=Gp[:, c0 : c0 + nb, :],
                    in_=gs[:, : nb * 2 * D].rearrange("p (j x) -> p j x", j=nb),
                )

            # phase-2/3 preloads (issued late so phase-1 hT DMAs go first)
            idx_t = cpool.tile([P, total_cols], i32)
            nc.sync.dma_start(out=idx_t[:], in_=gidx[:, :])
            hloc_sb = cpool.tile([P, NBLK * D], f32)
            nc.sync.dma_start(out=hloc_sb[:], in_=hloc[:, :])
            hTloc_sb = cpool.tile([D, NLOCP], f32)
            nc.sync.dma_start(out=hTloc_sb[:], in_=hTloc[:, :])

            # collapse phase-1 -> phase-2 deps into one barrier (sync-wait
            # slots on a single instruction are limited)
            tc.strict_bb_all_engine_barrier()

            # persistent phase-3 state (finished in overlapping sub-tails)
            bun_all = cpool.tile([P, NBLK * D], f32)
            ssq_all = cpool.tile([P, NBLK], f32)
            nrm_all = cpool.tile([P, NBLK], f32)
            rn_all = cpool.tile([P, NBLK], f32)

            _tail_done = [0]

            def _subtail(upto):
                t0 = _tail_done[0]
                if upto <= t0:
                    return
                nc.scalar.activation(
                    out=nrm_all[:, t0:upto],
                    in_=ssq_all[:, t0:upto],
                    func=mybir.ActivationFunctionType.Sqrt,
                )
                nc.vector.tensor_scalar_max(
                    nrm_all[:, t0:upto], nrm_all[:, t0:upto], EPS
                )
                nc.vector.reciprocal(rn_all[:, t0:upto], nrm_all[:, t0:upto])
                for b0 in range(t0, upto, OW):
                    nb = min(OW, upto - b0)
                    ow = owpool.tile([P, OW * D], f32, tag="ow")
                    for i in range(nb):
                        blk = b0 + i
                        ob = ow[:, i * D : (i + 1) * D]
                        nc.vector.tensor_scalar(
                            out=ob,
                            in0=bun_all[:, blk * D : (blk + 1) * D],
                            scalar1=rn_all[:, blk : blk + 1],
                            scalar2=0.0,
                            op0=mybir.AluOpType.mult,
                            op1=mybir.AluOpType.max,
                        )
                        nc.vector.tensor_tensor(
                            out=ob,
                            in0=ob,
                            in1=hloc_sb[:, blk * D : (blk + 1) * D],
                            op=mybir.AluOpType.add,
                        )
                    nc.sync.dma_start(
                        out=outp[:, b0 * D : (b0 + nb) * D],
                        in_=ow[:, : nb * D],
                    )
                _tail_done[0] = upto

            # ---- phase 2+3 per local node block
            for blk in range(NBLK):
                kb = int(K[blk])
                o = int(offs[blk])

                gt = gatpool.tile([P, (kb + 1) * 2 * D], bf16, tag="gt")
                nc.gpsimd.indirect_dma_start(
                    out=gt[:],
                    out_offset=None,
                    in_=G[:, :],
                    in_offset=bass.IndirectOffsetOnAxis(
                        ap=idx_t[:, o : o + kb + 1], axis=0
                    ),
                )

                gv = gt[:, : kb * 2 * D].rearrange("p (k x) -> p k x", k=kb)
                ahs = gv[:, :, 0:D]
                bhs = gv[:, :, D : 2 * D]
                bhd = (
                    gt[:, kb * 2 * D + D : (kb + 1) * 2 * D]
                    .rearrange("p (o x) -> p o x", o=1)
                    .to_broadcast([P, kb, D])
                )

                s = sgpool.tile([P, kb * D], bf16, tag="s")
                s3 = s[:].rearrange("p (k x) -> p k x", k=kb)
                nc.vector.tensor_tensor(
                    out=s3, in0=bhs, in1=bhd, op=mybir.AluOpType.add
                )
                nc.scalar.activation(
                    out=s[:], in_=s[:], func=mybir.ActivationFunctionType.Sigmoid
                )
                nc.vector.tensor_tensor(
                    out=s3, in0=s3, in1=ahs, op=mybir.AluOpType.mult
                )

                # segment max: contiguous in-place tree of tensor_tensor(max)
                cb = p3pool.tile([P, D], f32, tag="cb")
                n = kb
                while n > 1:
                    if n & 1:
                        # fold the tail column into column 0, making n even
                        nc.vector.tensor_tensor(
                            out=s[:, 0:D],
                            in0=s[:, 0:D],
                            in1=s[:, (n - 1) * D : n * D],
                            op=mybir.AluOpType.max,
                        )
                        n -= 1
                        if n == 1:
                            break
                    h = n // 2
                    if h == 1:
                        break
                    nc.vector.tensor_tensor(
                        out=s[:, : h * D],
                        in0=s[:, : h * D],
                        in1=s[:, h * D : n * D],
                        op=mybir.AluOpType.max,
                    )
                    n = h
                if n == 1:
                    nc.vector.tensor_scalar_max(cb[:], s[:, 0:D], 0.0)
                else:
                    # cb = max(s0, s1, 0): relu folded into the last tree op
                    nc.vector.scalar_tensor_tensor(
                        out=cb[:],
                        in0=s[:, 0:D],
                        scalar=0.0,
                        in1=s[:, D : 2 * D],
                        op0=mybir.AluOpType.max,
                        op1=mybir.AluOpType.max,
                    )

                # ---- phase 3 (fused): bundle = h @ U1 + c @ U2 + U_b
                ct_ps_t = ps3pool.tile([P, D], f32, tag="ctps")
                ct_ps = ct_ps_t[:]
                bp_t = ps3pool.tile([P, D], f32, tag="bp")
                bp = bp_t[:]
                nc.tensor.transpose(out=ct_ps, in_=cb[:], identity=ident[:])
                ct = p3pool.tile([P, D], f32, tag="ct")
                nc.scalar.copy(out=ct[:], in_=ct_ps)

                nc.tensor.matmul(
                    out=bp,
                    lhsT=hTloc_sb[:, blk * P : (blk + 1) * P],
                    rhs=u1_t[:],
                    start=True,
                    stop=False,
                )
                nc.tensor.matmul(
                    out=bp, lhsT=ct[:], rhs=u2_t[:], start=False, stop=True
                )

                # bundle = bp + U_b, moved to SBUF (VectorE)
                bun = bun_all[:, blk * D : (blk + 1) * D]
                nc.vector.tensor_tensor(
                    out=bun, in0=bp, in1=ub_sb[:], op=mybir.AluOpType.add
                )
                # sum of squares (ScalarE Square is in the sigmoid table set)
                sq = p3pool.tile([P, D], f32, tag="sq")
                nc.scalar.activation(
                    out=sq[:],
                    in_=bun,
                    func=mybir.ActivationFunctionType.Square,
                    accum_out=ssq_all[:, blk : blk + 1],
                )

            # ---- final sub-tail flush
            _subtail(NBLK)

    nc.compile()
    return nc


# --------------------------------------------------------------------------- run


def _run(inputs, trace=False):
    h = np.asarray(inputs["h"], np.float32)
    A_w = np.asarray(inputs["A_w"], np.float32)
    A_b = np.asarray(inputs["A_b"], np.float32)
    B_w = np.asarray(inputs["B_w"], np.float32)
    B_b = np.asarray(inputs["B_b"], np.float32)
    U_w = np.asarray(inputs["U_w"], np.float32)
    U_b = np.asarray(inputs["U_b"], np.float32)

    K, offs, total_cols, per_core = _preprocess(h, inputs["src"], inputs["dst"])

    key = (tuple(int(k) for k in K), total_cols)
    if key not in _prog_cache:
        _prog_cache.clear()
        _prog_cache[key] = _build(K, offs, total_cols)
    nc = _prog_cache[key]

    hT_bf = np.zeros((D, NPADG), BF16)
    hT_bf[:, :N] = h.T.astype(BF16)
    wcat = np.concatenate([A_w, B_w], axis=1).astype(BF16)
    brhs = np.tile(np.concatenate([A_b, B_b]), PC1)[None, :].astype(np.float32)
    u1 = np.ascontiguousarray(U_w[:D]).astype(np.float32)
    u2 = np.ascontiguousarray(U_w[D:]).astype(np.float32)
    ubr = U_b[None, :].astype(np.float32)

    in_maps = []
    for c in range(NCORES):
        gidx_c, perm = per_core[c]
        hl = np.zeros((NLOCP, D), np.float32)
        hl[:NLOC] = h[c * NLOC + perm]
        # partition-major [p, blk, x] layout for contiguous per-partition DMA
        hl_p = np.ascontiguousarray(
            hl.reshape(NBLK, P, D).transpose(1, 0, 2).reshape(P, NBLK * D)
        )
        in_maps.append(
            {
                "hT": hT_bf,
                "hloc": hl_p,
                "hTloc": np.ascontiguousarray(hl.T),
                "gidx": gidx_c,
                "wcat": wcat,
                "brhs": brhs,
                "u1": u1,
                "u2": u2,
                "ubr": ubr,
            }
        )

    res = run_bass_kernel_spmd(nc, in_maps, list(range(NCORES)), trace=trace)

    out = np.empty((N, D), np.float32)
    for c in range(NCORES):
        _, perm = per_core[c]
        # undo partition-major layout: res [p, blk, x] -> [blk*P + p, x]
        o = (
            res.results[c]["out"]
            .reshape(P, NBLK, D)
            .transpose(1, 0, 2)
            .reshape(NLOCP, D)
        )
        out[c * NLOC + perm] = o[:NLOC]
    return out, res


def kernel(**inputs) -> np.ndarray:
    out, _ = _run(inputs, trace=False)
    return out


# revision 20
# speedup vs baseline: 1.2290x; 1.1068x over previous
"""GraphSAGE edge layer (sigmoid-gated message passing + segment-max) on 8 Trainium2
NeuronCores via Bass/Tile.

Strategy (graph/data parallel):
  - Nodes are sharded across 8 cores (6250 each); edges partitioned by destination
    node so the segment-max reduce is core-local.
  - Per core, a replicated G table [Ah | Bh] (bf16) is built on-device in a
    partition-major layout: node n lives at row (n%128)*NBLK1 + n//128, so each
    partition's G rows are contiguous and phase-1 writes are large descriptors.
  - Edges are packed host-side into a degree-bucketed (node x slot) grid: 49 blocks
    of 128 nodes, block b has K_b slots/node (max in-degree in block, shared across
    cores so the SPMD program has one shape schedule). Pad slots point at a zero row
    of G, whose Ah half is 0 => gated contribution is exactly 0, which is absorbed
    by the final clamp-to-0 (relu of the segment max).
  - Per block: ONE indirect DMA gathers all slot rows (+ the block's own rows for
    Bh[dst]); VectorE/ScalarE compute sigmoid(BhS + BhD) * AhS; the segment max is
    a contiguous in-place tree of tensor_tensor(max) ops (2x bf16 DVE mode); the
    update MLP runs fused per block in f32; sqrt/normalize/residual for all blocks
    run in one deferred tail pass (single activation-table load, batched stores).
"""

import numpy as np
import ml_dtypes

from concourse import bass, bacc, mybir
from concourse.tile import TileContext
from concourse.bass_utils import run_bass_kernel_spmd
from concourse.masks import make_identity

BF16 = ml_dtypes.bfloat16

N = 50000
E = 800000
D = 128
NCORES = 8
NLOC = N // NCORES          # 6250 nodes per core
P = 128
NBLK1 = (N + P - 1) // P    # 391 G-build blocks
NPADG = NBLK1 * P           # 50048 (pad/zero row of G lives at this index)
NBLK = (NLOC + P - 1) // P  # 49 local node blocks
NLOCP = NBLK * P            # 6272
EPS = 1e-12
CH1 = 18                    # G-build hT chunk (blocks per DMA)
PC1 = 6                     # G-build PSUM chunk (blocks per PSUM tile)
OW = 4                      # output write chunk (blocks per DMA)

_prog_cache = {}


# --------------------------------------------------------------------------- host


def _rowmap(n):
    """Node id -> partition-major G row."""
    return (n % P) * NBLK1 + n // P


def _preprocess(h, src, dst):
    """Shard edges by destination, build per-core degree-sorted slot grids.

    Returns (K, offs, total_cols, per_core) where per_core[c] =
    (gidx [128, total_cols] int32, perm [NLOC] int64). gidx entries are
    partition-major G row indices (or NPADG for pad slots).
    """
    src = np.asarray(src).astype(np.int64)
    dst = np.asarray(dst).astype(np.int64)

    order = np.argsort(dst, kind="stable")
    dst_s = dst[order]
    src_s = src[order]
    bounds = np.searchsorted(dst_s, np.arange(NCORES + 1) * NLOC)

    cores = []
    blkmax = np.zeros((NCORES, NBLK), np.int64)
    for c in range(NCORES):
        a, b = bounds[c], bounds[c + 1]
        ldst = dst_s[a:b] - c * NLOC
        lsrc = src_s[a:b]
        deg = np.bincount(ldst, minlength=NLOC)
        perm = np.argsort(-deg, kind="stable")
        pdeg = np.zeros(NLOCP, np.int64)
        pdeg[:NLOC] = deg[perm]
        blkmax[c] = pdeg.reshape(NBLK, P).max(axis=1)
        cores.append((lsrc, deg, perm))

    K = np.maximum(blkmax.max(axis=0), 1).astype(np.int64)
    offs = np.concatenate([[0], np.cumsum(K + 1)]).astype(np.int64)
    total_cols = int(offs[-1])

    per_core = []
    for c in range(NCORES):
        lsrc, deg, perm = cores[c]
        start = np.concatenate([[0], np.cumsum(deg)])
        gidx = np.full((P, total_cols), NPADG, np.int32)
        for blk in range(NBLK):
            o = int(offs[blk])
            kb = int(K[blk])
            for p in range(P):
                i = blk * P + p
                if i >= NLOC:
                    continue
                node = int(perm[i])
                d = int(deg[node])
                if d:
                    e = lsrc[start[node] : start[node] + d]
                    gidx[p, o : o + d] = _rowmap(e)
                gidx[p, o + kb] = _rowmap(c * NLOC + node)
        per_core.append((gidx, perm))
    return K, offs, total_cols, per_core


# --------------------------------------------------------------------------- bass


def _build(K, offs, total_cols):
    f32 = mybir.dt.float32
    bf16 = mybir.dt.bfloat16
    i32 = mybir.dt.int32

    nc = bacc.Bacc(
        "TRN2", target_bir_lowering=False, debug=False, num_devices=NCORES
    )
    hT = nc.declare_dram_parameter("hT", [D, NPADG], bf16, isOutput=False)
    # hloc is partition-major: [p, blk, x]
    hloc = nc.declare_dram_parameter("hloc", [P, NBLK * D], f32, isOutput=False)
    hTloc = nc.declare_dram_parameter("hTloc", [D, NLOCP], f32, isOutput=False)
    gidx = nc.declare_dram_parameter("gidx", [P, total_cols], i32, isOutput=False)
    wcat = nc.declare_dram_parameter("wcat", [D, 2 * D], bf16, isOutput=False)
    brhs = nc.declare_dram_parameter("brhs", [1, PC1 * 2 * D], f32, isOutput=False)
    u1 = nc.declare_dram_parameter("u1", [D, D], f32, isOutput=False)
    u2 = nc.declare_dram_parameter("u2", [D, D], f32, isOutput=False)
    ubr = nc.declare_dram_parameter("ubr", [1, D], f32, isOutput=False)
    # out is partition-major: [p, blk, x]
    outp = nc.declare_dram_parameter("out", [P, NBLK * D], f32, isOutput=True)

    G = nc.dram_tensor("G", [NPADG + 1, 2 * D], bf16)
    # partition-major view of the first NPADG rows: row (p*NBLK1 + j)
    Gp = G[0:NPADG, :].rearrange("(p j) x -> p j x", p=P)

    with TileContext(nc) as tc:
        with (
            tc.tile_pool(name="const", bufs=1) as cpool,
            tc.tile_pool(name="ht1", bufs=2) as htpool,
            tc.tile_pool(name="gs1", bufs=2) as gspool,
            tc.tile_pool(name="gat", bufs=3) as gatpool,
            tc.tile_pool(name="sg", bufs=2) as sgpool,
            tc.tile_pool(name="p3", bufs=3) as p3pool,
            tc.tile_pool(name="ow", bufs=2) as owpool,
            tc.tile_pool(name="ps1", bufs=2, space="PSUM") as ps1pool,
            tc.tile_pool(name="ps3", bufs=2, space="PSUM") as ps3pool,
        ):
            # ---- constants
            wcat_t = cpool.tile([D, 2 * D], bf16)
            nc.sync.dma_start(out=wcat_t[:], in_=wcat[:, :])
            brhs_t = cpool.tile([1, PC1 * 2 * D], f32)
            nc.sync.dma_start(out=brhs_t[:], in_=brhs[:, :])
            u1_t = cpool.tile([D, D], f32)
            nc.sync.dma_start(out=u1_t[:], in_=u1[:, :])
            u2_t = cpool.tile([D, D], f32)
            nc.sync.dma_start(out=u2_t[:], in_=u2[:, :])
            ubr_t = cpool.tile([1, D], f32)
            nc.sync.dma_start(out=ubr_t[:], in_=ubr[:, :])
            ident = cpool.tile([P, P], f32)
            make_identity(nc, ident[:])

            # bias rows replicated across partitions (matmul ones-trick)
            ones_f32 = cpool.tile([1, P], f32)
            nc.vector.memset(ones_f32[:], 1.0)
            bias_ps = ps1pool.tile([P, PC1 * 2 * D], f32, tag="gps")
            for q in range(0, PC1 * 2 * D, 512):
                nc.tensor.matmul(
                    out=bias_ps[:, q : q + 512],
                    lhsT=ones_f32[:],
                    rhs=brhs_t[:, q : q + 512],
                    start=True,
                    stop=True,
                )
            bias6_sb = cpool.tile([P, PC1 * 2 * D], f32)
            nc.scalar.copy(out=bias6_sb[:], in_=bias_ps[:])


            zrow = cpool.tile([1, 2 * D], bf16)
            nc.vector.memset(zrow[:], 0.0)
            nc.sync.dma_start(out=G[NPADG : NPADG + 1, :], in_=zrow[:])

            # ---- phase 1: G = [h @ A_w + A_b | h @ B_w + B_b]  (bf16)
            for c0 in range(0, NBLK1, CH1):
                nb = min(CH1, NBLK1 - c0)
                ht_t = htpool.tile([D, CH1 * P], bf16, tag="ht")
                nc.sync.dma_start(
                    out=ht_t[:, : nb * P],
                    in_=hT[:, c0 * P : (c0 + nb) * P],
                )
                gs = gspool.tile([P, CH1 * 2 * D], bf16, tag="gs")
                for g0 in range(0, nb, PC1):
                    gn = min(PC1, nb - g0)
                    ps = ps1pool.tile([P, PC1 * 2 * D], f32, tag="gps")
                    for j in range(g0, g0 + gn):
                        nc.tensor.matmul(
                            out=ps[:, (j - g0) * 2 * D : (j - g0 + 1) * 2 * D],
                            lhsT=ht_t[:, j * P : (j + 1) * P],
                            rhs=wcat_t[:],
                            start=True,
                            stop=True,
                        )
                    # chunked bias add fused with PSUM -> SBUF move (VectorE)
                    nc.vector.tensor_tensor(
                        out=gs[:, g0 * 2 * D : (g0 + gn) * 2 * D],
                        in0=ps[:, : gn * 2 * D],
                        in1=bias6_sb[:, : gn * 2 * D],
                        op=mybir.AluOpType.add,
                    )
                nc.sync.dma_start(
                    out=Gp[:, c0 : c0 + nb, :],
                    in_=gs[:, : nb * 2 * D].rearrange("p (j x) -> p j x", j=nb),
                )

            # phase-2/3 preloads (issued late so phase-1 hT DMAs go first)
            idx_t = cpool.tile([P, total_cols], i32)
            nc.sync.dma_start(out=idx_t[:], in_=gidx[:, :])
            hloc_sb = cpool.tile([P, NBLK * D], f32)
            nc.sync.dma_start(out=hloc_sb[:], in_=hloc[:, :])
            hTloc_sb = cpool.tile([D, NLOCP], f32)
            nc.sync.dma_start(out=hTloc_sb[:], in_=hTloc[:, :])

            # collapse phase-1 -> phase-2 deps into one barrier (sync-wait
            # slots on a single instruction are limited)
            tc.strict_bb_all_engine_barrier()

            # persistent phase-3 state (finished in overlapping sub-tails)
            bun_all = cpool.tile([P, NBLK * D], f32)
            ssq_all = cpool.tile([P, NBLK], f32)
            nrm_all = cpool.tile([P, NBLK], f32)
            rn_all = cpool.tile([P, NBLK], f32)

            _tail_done = [0]

            def _subtail(upto):
                t0 = _tail_done[0]
                if upto <= t0:
                    return
                nc.scalar.activation(
                    out=nrm_all[:, t0:upto],
                    in_=ssq_all[:, t0:upto],
                    func=mybir.ActivationFunctionType.Sqrt,
                )
                nc.vector.tensor_scalar_max(
                    nrm_all[:, t0:upto], nrm_all[:, t0:upto], EPS
                )
                nc.vector.reciprocal(rn_all[:, t0:upto], nrm_all[:, t0:upto])
                for b0 in range(t0, upto, OW):
                    nb = min(OW, upto - b0)
                    ow = owpool.tile([P, OW * D], f32, tag="ow")
                    for i in range(nb):
                        blk = b0 + i
                        ob = ow[:, i * D : (i + 1) * D]
                        nc.vector.tensor_scalar(
                            out=ob,
                            in0=bun_all[:, blk * D : (blk + 1) * D],
                            scalar1=rn_all[:, blk : blk + 1],
                            scalar2=0.0,
                            op0=mybir.AluOpType.mult,
                            op1=mybir.AluOpType.max,
                        )
                        nc.vector.tensor_tensor(
                            out=ob,
                            in0=ob,
                            in1=hloc_sb[:, blk * D : (blk + 1) * D],
                            op=mybir.AluOpType.add,
                        )
                    nc.sync.dma_start(
                        out=outp[:, b0 * D : (b0 + nb) * D],
                        in_=ow[:, : nb * D],
                    )
                _tail_done[0] = upto

            # ---- phase 2+3 per local node block
            for blk in range(NBLK):
                kb = int(K[blk])
                o = int(offs[blk])

                gt = gatpool.tile([P, (kb + 1) * 2 * D], bf16, tag="gt")
                nc.gpsimd.indirect_dma_start(
                    out=gt[:],
                    out_offset=None,
                    in_=G[:, :],
                    in_offset=bass.IndirectOffsetOnAxis(
                        ap=idx_t[:, o : o + kb + 1], axis=0
                    ),
                )

                gv = gt[:, : kb * 2 * D].rearrange("p (k x) -> p k x", k=kb)
                ahs = gv[:, :, 0:D]
                bhs = gv[:, :, D : 2 * D]
                bhd = (
                    gt[:, kb * 2 * D + D : (kb + 1) * 2 * D]
                    .rearrange("p (o x) -> p o x", o=1)
                    .to_broadcast([P, kb, D])
                )

                s = sgpool.tile([P, kb * D], bf16, tag="s")
                s3 = s[:].rearrange("p (k x) -> p k x", k=kb)
                nc.vector.tensor_tensor(
                    out=s3, in0=bhs, in1=bhd, op=mybir.AluOpType.add
                )
                nc.scalar.activation(
                    out=s[:], in_=s[:], func=mybir.ActivationFunctionType.Sigmoid
                )
                nc.vector.tensor_tensor(
                    out=s3, in0=s3, in1=ahs, op=mybir.AluOpType.mult
                )

                # segment max: contiguous in-place tree of tensor_tensor(max)
                cb = p3pool.tile([P, D], f32, tag="cb")
                n = kb
                first_level = True
                while n > 1:
                    if n & 1:
                        # fold the tail column into column 0, making n even
                        nc.vector.tensor_tensor(
                            out=s[:, 0:D],
                            in0=s[:, 0:D],
                            in1=s[:, (n - 1) * D : n * D],
                            op=mybir.AluOpType.max,
                        )
                        n -= 1
                        if n == 1:
                            break
                    h = n // 2
                    if h == 1:
                        break
                    nc.vector.tensor_tensor(
                        out=s[:, : h * D],
                        in0=s[:, : h * D],
                        in1=s[:, h * D : n * D],
                        op=mybir.AluOpType.max,
                    )
                    n = h
                if n == 1:
                    nc.vector.tensor_scalar_max(cb[:], s[:, 0:D], 0.0)
                else:
                    # cb = max(s0, s1, 0): relu folded into the last tree op
                    nc.vector.scalar_tensor_tensor(
                        out=cb[:],
                        in0=s[:, 0:D],
                        scalar=0.0,
                        in1=s[:, D : 2 * D],
                        op0=mybir.AluOpType.max,
                        op1=mybir.AluOpType.max,
                    )

                # ---- phase 3 (fused): bundle = h @ U1 + c @ U2 + U_b
                p3ps = ps3pool.tile([P, 2 * D], f32, tag="p3ps")
                ct_ps = p3ps[:, 0:D]
                bp = p3ps[:, D : 2 * D]
                nc.tensor.transpose(out=ct_ps, in_=cb[:], identity=ident[:])
                ct = p3pool.tile([P, D], f32, tag="ct")
                nc.scalar.copy(out=ct[:], in_=ct_ps)

                nc.tensor.matmul(
                    out=bp, lhsT=ones_f32[:], rhs=ubr_t[:], start=True, stop=False
                )
                nc.tensor.matmul(
                    out=bp,
                    lhsT=hTloc_sb[:, blk * P : (blk + 1) * P],
                    rhs=u1_t[:],
                    start=False,
                    stop=False,
                )
                nc.tensor.matmul(
                    out=bp, lhsT=ct[:], rhs=u2_t[:], start=False, stop=True
                )

                # bundle moved to SBUF (ScalarE)
                bun = bun_all[:, blk * D : (blk + 1) * D]
                nc.scalar.copy(out=bun, in_=bp)
                # sum of squares (ScalarE Square is in the sigmoid table set)
                sq = p3pool.tile([P, D], f32, tag="sq")
                nc.scalar.activation(
                    out=sq[:],
                    in_=bun,
                    func=mybir.ActivationFunctionType.Square,
                    accum_out=ssq_all[:, blk : blk + 1],
                )

            # ---- final sub-tail flush
            _subtail(NBLK)

    nc.compile()
    return nc


# --------------------------------------------------------------------------- run


def _run(inputs, trace=False):
    h = np.asarray(inputs["h"], np.float32)
    A_w = np.asarray(inputs["A_w"], np.float32)
    A_b = np.asarray(inputs["A_b"], np.float32)
    B_w = np.asarray(inputs["B_w"], np.float32)
    B_b = np.asarray(inputs["B_b"], np.float32)
    U_w = np.asarray(inputs["U_w"], np.float32)
    U_b = np.asarray(inputs["U_b"], np.float32)

    K, offs, total_cols, per_core = _preprocess(h, inputs["src"], inputs["dst"])

    key = (tuple(int(k) for k in K), total_cols)
    if key not in _prog_cache:
        _prog_cache.clear()
        _prog_cache[key] = _build(K, offs, total_cols)
    nc = _prog_cache[key]

    hT_bf = np.zeros((D, NPADG), BF16)
    hT_bf[:, :N] = h.T.astype(BF16)
    wcat = np.concatenate([A_w, B_w], axis=1).astype(BF16)
    brhs = np.tile(np.concatenate([A_b, B_b]), PC1)[None, :].astype(np.float32)
    u1 = np.ascontiguousarray(U_w[:D]).astype(np.float32)
    u2 = np.ascontiguousarray(U_w[D:]).astype(np.float32)
    ubr = U_b[None, :].astype(np.float32)

    in_maps = []
    for c in range(NCORES):
        gidx_c, perm = per_core[c]
        hl = np.zeros((NLOCP, D), np.float32)
        hl[:NLOC] = h[c * NLOC + perm]
        # partition-major [p, blk, x] layout for contiguous per-partition DMA
        hl_p = np.ascontiguousarray(
            hl.reshape(NBLK, P, D).transpose(1, 0, 2).reshape(P, NBLK * D)
        )
        in_maps.append(
            {
                "hT": hT_bf,
                "hloc": hl_p,
                "hTloc": np.ascontiguousarray(hl.T),
                "gidx": gidx_c,
                "wcat": wcat,
                "brhs": brhs,
                "u1": u1,
                "u2": u2,
                "ubr": ubr,
            }
        )

    res = run_bass_kernel_spmd(nc, in_maps, list(range(NCORES)), trace=trace)

    out = np.empty((N, D), np.float32)
    for c in range(NCORES):
        _, perm = per_core[c]
        # undo partition-major layout: res [p, blk, x] -> [blk*P + p, x]
        o = (
            res.results[c]["out"]
            .reshape(P, NBLK, D)
            .transpose(1, 0, 2)
            .reshape(NLOCP, D)
        )
        out[c * NLOC + perm] = o[:NLOC]
    return out, res


def kernel(**inputs) -> np.ndarray:
    out, _ = _run(inputs, trace=False)
    return out
